# revision 1
# baseline (speedup 1.0000x reference)
"""Causal self-attention (B=2, T=2048, D=2048, H=16, HD=128) on 8 TRN2 cores.

Strategy: tensor-parallel over heads (2 heads/core) for QKV projection, RMS
norm, rotary, and attention; AllToAll reshards the attention output over
tokens; each core then runs the output projection for its 512-token slice.
All matmul contractions sit on the partition dim via host-side transposes:
  - qT/kT/vT come out of the QKV matmul as [outdim, token] tiles directly,
  - scores are computed transposed (S^T = krot^T.T @ qrot^T -> [k, q]), so
    the PV matmul needs no on-device transposes,
  - softmax denominator = all-ones matmul over expS^T (also acts as the
    partition-broadcast), normalization folds into the PSUM->SBUF copy.
Matmuls run in bf16 (fp32 is 4 cycles/row on the PE, bf16 is 1); PSUM
accumulation and softmax statistics stay fp32.
"""

import numpy as np

B, T, D = 2, 2048, 2048
H, HD = 16, 128
N_CORES = 8
HPC = H // N_CORES          # heads per core
NT = B * T                  # 4096 tokens, b-major
TS = NT // N_CORES          # 512-token output slice per core
DC = D // 128               # 16 contraction chunks
NTT = NT // 512             # 8 token tiles in phase 1
KT_PER_B = T // 128         # 16 k-tiles per batch row

_CACHE = {}


def _build(scale: float, reps: int = 1):
    import concourse.bacc as bacc
    import concourse.mybir as mybir
    import concourse.tile as tile

    f32 = mybir.dt.float32
    MM = mybir.dt.bfloat16
    EPS = float(np.finfo(np.float32).eps)

    nc = bacc.Bacc("TRN2", target_bir_lowering=False, debug=False,
                   num_devices=N_CORES)

    xT_d = nc.dram_tensor("xT", [D, NT], MM, kind="ExternalInput")
    wqk_d = nc.dram_tensor("wqk", [D, 4 * 128], MM, kind="ExternalInput")
    wv_d = nc.dram_tensor("wv", [D, HPC * HD], MM, kind="ExternalInput")
    wo_d = nc.dram_tensor("wo", [D, D], MM, kind="ExternalInput")
    cs_d = nc.dram_tensor("cs", [2, HD, NT], MM, kind="ExternalInput")
    mask_d = nc.dram_tensor("mask", [4, 128, 512], MM, kind="ExternalInput")
    ones_d = nc.dram_tensor("ones", [128, 128], MM, kind="ExternalInput")
    y_d = nc.dram_tensor("y", [TS, D], f32, kind="ExternalOutput")

    Sq = mybir.ActivationFunctionType.Square
    Sqrt = mybir.ActivationFunctionType.Sqrt
    Exp = mybir.ActivationFunctionType.Exp
    Copy = mybir.ActivationFunctionType.Copy
    mult = mybir.AluOpType.mult
    add = mybir.AluOpType.add

    with tile.TileContext(nc) as tc:
        with tc.tile_pool(name="dram", bufs=1, space="DRAM") as dram, \
             tc.tile_pool(name="res", bufs=1) as res:
            a2a_in_h = [dram.tile([N_CORES, HD, TS], MM, tag=f"a2a_in{h}",
                                  name=f"a2a_in{h}") for h in range(HPC)]
            a2a_out_h = [dram.tile([N_CORES, HD, TS], MM, tag=f"a2a_out{h}",
                                   name=f"a2a_out{h}") for h in range(HPC)]

            # Residents through phase 2: rotated q/k (m-chunks: q0,q1,k0,k1),
            # v in [token, hd] layout, causal masks, all-ones stationary.
            qk_sb = res.tile([128, 4 * NT], MM, tag="qk")
            v_sb = res.tile([128, (NT // 128) * (HPC * HD)], MM, tag="v")
            mask_sb = res.tile([128, 4 * 512], MM, tag="mask")
            ones_sb = res.tile([128, 128], MM, tag="ones")
            eps_sb = res.tile([128, 1], f32, tag="eps")
            nc.vector.memset(eps_sb[:], EPS)
            for m in range(4):
                nc.sync.dma_start(out=mask_sb[:, m * 512:(m + 1) * 512],
                                  in_=mask_d[m, :, :])
            nc.sync.dma_start(out=ones_sb[:], in_=ones_d[:, :])

            for _rep in range(reps):
                # ---------------- Phase 1: QKV + RMS norm + rotary ----------------
                with tc.tile_pool(name="p1", bufs=1) as p1, \
                     tc.tile_pool(name="xs", bufs=3) as xs, \
                     tc.tile_pool(name="st", bufs=3) as st, \
                     tc.tile_pool(name="ps1", bufs=2, space="PSUM") as ps1:
                    wqk_sb = p1.tile([128, DC * 512], MM, tag="wqk")
                    wv_sb = p1.tile([128, DC * HPC * HD], MM, tag="wv")
                    cs_sb = p1.tile([128, 2 * NT], MM, tag="cs")
                    nc.sync.dma_start(
                        out=wv_sb[:].rearrange("p (c f) -> p c f", f=256),
                        in_=wv_d[:, :].rearrange("(c p) f -> p c f", p=128))

                    for n in range(NTT):
                        xblk = xs.tile([128, DC * 512], MM, tag="xblk")
                        for cg in range(4):
                            nc.sync.dma_start(
                                out=xblk[:, cg * 4 * 512:(cg + 1) * 4 * 512]
                                    .rearrange("p (c f) -> p c f", f=512),
                                in_=xT_d[cg * 512:(cg + 1) * 512, n * 512:(n + 1) * 512]
                                    .rearrange("(c p) f -> p c f", p=128))
                        # v projection: [token, hd] layout
                        for c4 in range(4):
                            vps = ps1.tile([128, HPC * HD], f32, tag="vps")
                            for dc in range(DC):
                                nc.tensor.matmul(
                                    vps[:],
                                    xblk[:, dc * 512 + c4 * 128: dc * 512 + (c4 + 1) * 128],
                                    wv_sb[:, dc * 256:(dc + 1) * 256],
                                    start=(dc == 0), stop=(dc == DC - 1))
                            tcg = n * 4 + c4
                            nc.vector.tensor_copy(v_sb[:, tcg * 256:(tcg + 1) * 256], vps[:])
                        if n == 0:
                            # emitted late so the first v chain's loads go first
                            nc.sync.dma_start(
                                out=wqk_sb[:].rearrange("p (c f) -> p c f", f=512),
                                in_=wqk_d[:, :].rearrange("(c p) f -> p c f", p=128))
                            for s in range(2):
                                nc.sync.dma_start(out=cs_sb[:, s * NT:(s + 1) * NT],
                                                  in_=cs_d[s, :, :])
                        # q/k projection + rms + rotary, m-chunks q0,q1,k0,k1
                        for m in range(4):
                            qps = ps1.tile([128, 512], f32, tag="qps")
                            for dc in range(DC):
                                nc.tensor.matmul(
                                    qps[:],
                                    wqk_sb[:, dc * 512 + m * 128: dc * 512 + (m + 1) * 128],
                                    xblk[:, dc * 512:(dc + 1) * 512],
                                    start=(dc == 0), stop=(dc == DC - 1))
                            sq = st.tile([128, 512], MM, tag="sq")
                            nc.scalar.activation(sq[:], qps[:], Sq)
                            ssq = ps1.tile([128, 512], f32, tag="ssq")
                            nc.tensor.matmul(ssq[:], ones_sb[:], sq[:], start=True, stop=True)
                            rms = st.tile([128, 512], f32, tag="rms")
                            nc.scalar.activation(rms[:], ssq[:], Sqrt, bias=eps_sb[:], scale=1.0 / HD)
                            r = st.tile([128, 512], f32, tag="r")
                            nc.vector.reciprocal(r[:], rms[:])
                            qn = st.tile([128, 512], MM, tag="qn")
                            nc.vector.tensor_mul(qn[:], qps[:], r[:])
                            # rotary: y = qn*C + swap(qn)*S  with S = [sin; -sin]
                            tsw = st.tile([128, 512], MM, tag="tsw")
                            ctile = cs_sb[:, n * 512:(n + 1) * 512]
                            stile = cs_sb[:, NT + n * 512: NT + (n + 1) * 512]
                            # stile holds [-sin; sin]: each mul's inputs share a
                            # base partition; only the output is partition-shifted.
                            nc.vector.tensor_mul(tsw[0:64, :], qn[64:128, :], stile[64:128, :])
                            nc.vector.tensor_mul(tsw[64:128, :], qn[0:64, :], stile[0:64, :])
                            dst = qk_sb[:, m * NT + n * 512: m * NT + (n + 1) * 512]
                            nc.vector.tensor_mul(dst, qn[:], ctile)
                            nc.vector.tensor_add(dst, dst, tsw[:])

                # ---------------- Phase 2 + 3: attention, A2A, o-proj ----------------
                # h is the outer loop so head 0's AllToAll overlaps head 1's
                # attention; o-proj accumulates even d-chunks (head 0 data) first
                # so it can start before the second AllToAll lands.
                with tc.tile_pool(name="p2", bufs=4) as p2, \
                     tc.tile_pool(name="p2b", bufs=2) as p2b, \
                     tc.tile_pool(name="pss", bufs=2, space="PSUM") as pss, \
                     tc.tile_pool(name="psd", bufs=2, space="PSUM") as psd, \
                     tc.tile_pool(name="psy", bufs=2, space="PSUM") as psy, \
                     tc.tile_pool(name="p3", bufs=1) as p3, \
                     tc.tile_pool(name="wop", bufs=4) as wop, \
                     tc.tile_pool(name="ob", bufs=2) as obp, \
                     tc.tile_pool(name="prt", bufs=16) as prt, \
                     tc.tile_pool(name="ps3", bufs=2, space="PSUM") as ps3:
                    for h in range(HPC):
                        qoff = h * NT
                        koff = (2 + h) * NT
                        for b in range(B):
                            for qj in range(4):
                                yps = psy.tile([128, 512], f32, tag="yps")
                                dps = psd.tile([128, 512], f32, tag="dps")
                                nkt = 4 * qj + 4
                                qbase = qoff + b * T + qj * 512
                                for kb in range(nkt):
                                    # diagonal blocks: only q-columns >= 128*m live
                                    lo = max(0, (kb - 4 * qj) * 128)
                                    sps = pss.tile([128, 512], f32, tag="sps")
                                    nc.tensor.matmul(
                                        sps[:, lo:],
                                        qk_sb[:, koff + b * T + kb * 128: koff + b * T + (kb + 1) * 128],
                                        qk_sb[:, qbase + lo: qbase + 512],
                                        start=True, stop=True)
                                    e = p2.tile([128, 512], MM, tag="e")
                                    nc.scalar.activation(e[:, lo:], sps[:, lo:], Exp, scale=scale)
                                    if kb >= 4 * qj:
                                        mi = kb - 4 * qj
                                        nc.vector.tensor_mul(
                                            e[:, lo:], e[:, lo:],
                                            mask_sb[:, mi * 512 + lo:(mi + 1) * 512])
                                    nc.tensor.matmul(dps[:, lo:], ones_sb[:], e[:, lo:],
                                                     start=(kb == 0), stop=(kb == nkt - 1))
                                    tcg = b * KT_PER_B + kb
                                    nc.tensor.matmul(
                                        yps[:, lo:],
                                        v_sb[:, tcg * 256 + h * 128: tcg * 256 + (h + 1) * 128],
                                        e[:, lo:],
                                        start=(kb == 0), stop=(kb == nkt - 1))
                                rcp = p2b.tile([128, 512], f32, tag="rcp")
                                nc.vector.reciprocal(rcp[:], dps[:])
                                yn = p2b.tile([128, 512], MM, tag="yn")
                                nc.vector.tensor_mul(yn[:], yps[:], rcp[:])
                                s = b * 4 + qj
                                nc.sync.dma_start(out=a2a_in_h[h][s, :, :], in_=yn[:])
                        nc.gpsimd.collective_compute(
                            "AllToAll",
                            mybir.AluOpType.bypass,
                            replica_groups=[list(range(N_CORES))],
                            ins=[a2a_in_h[h].opt()],
                            outs=[a2a_out_h[h].opt()],
                        )

                    # o-proj: d-chunk dc2 = 2g + h lives in a2a_out_h[h][g];
                    # per-g DMAs so early chains need not wait for the full load
                    yT_h = []
                    for h in range(HPC):
                        yt = p3.tile([128, N_CORES * 512], MM, tag=f"yT{h}",
                                     name=f"yT{h}")
                        for g in range(N_CORES):
                            nc.sync.dma_start(out=yt[:, g * 512:(g + 1) * 512],
                                              in_=a2a_out_h[h][g, :, :])
                        yT_h.append(yt)
                    # all even (head-0) chains first, then all odd chains:
                    # keeps PSUM slot reuse from chaining evens behind odds
                    # that wait on the second collective
                    wo_blocks = []
                    for on in range(4):
                        wo_sb = wop.tile([128, DC * 512], MM, tag="wo")
                        for cg in range(4):
                            nc.sync.dma_start(
                                out=wo_sb[:, cg * 4 * 512:(cg + 1) * 4 * 512]
                                    .rearrange("p (c f) -> p c f", f=512),
                                in_=wo_d[cg * 512:(cg + 1) * 512, on * 512:(on + 1) * 512]
                                    .rearrange("(c p) f -> p c f", p=128))
                        wo_blocks.append(wo_sb)
                    parts = []
                    for on in range(4):
                        for mc in range(4):
                            pe_ps = ps3.tile([128, 512], f32, tag="ops")
                            for g in range(8):
                                nc.tensor.matmul(
                                    pe_ps[:],
                                    yT_h[0][:, g * 512 + mc * 128: g * 512 + (mc + 1) * 128],
                                    wo_blocks[on][:, 2 * g * 512:(2 * g + 1) * 512],
                                    start=(g == 0), stop=(g == 7))
                            part = prt.tile([128, 512], f32, tag="part")
                            nc.scalar.activation(part[:], pe_ps[:], Copy)
                            parts.append(part)
                    for on in range(4):
                        for mc in range(4):
                            po_ps = ps3.tile([128, 512], f32, tag="ops")
                            for g in range(8):
                                nc.tensor.matmul(
                                    po_ps[:],
                                    yT_h[1][:, g * 512 + mc * 128: g * 512 + (mc + 1) * 128],
                                    wo_blocks[on][:, (2 * g + 1) * 512:(2 * g + 2) * 512],
                                    start=(g == 0), stop=(g == 7))
                            ob = obp.tile([128, 512], f32, tag="ob")
                            nc.vector.tensor_add(ob[:], po_ps[:], parts[on * 4 + mc][:])
                            nc.sync.dma_start(
                                out=y_d[mc * 128:(mc + 1) * 128, on * 512:(on + 1) * 512],
                                in_=ob[:])

    nc.compile()
    return nc


def _prep_inputs(x, W, cos, sin):
    import concourse.mybir as mybir
    bf = mybir.dt.np(mybir.dt.bfloat16)

    xT = np.ascontiguousarray(x.reshape(NT, D).T).astype(bf)
    cT = cos.T.astype(np.float32)
    sT = sin.T.astype(np.float32)
    C128 = np.tile(np.concatenate([cT, cT], 0), (1, B)).astype(bf)
    S128 = np.tile(np.concatenate([-sT, sT], 0), (1, B)).astype(bf)
    cs = np.ascontiguousarray(np.stack([C128, S128]))
    masks = np.stack([
        (np.arange(128)[:, None] <= np.arange(512)[None, :] - 128 * m)
        for m in range(4)
    ]).astype(bf)
    ones = np.ones((128, 128), dtype=bf)
    wo = np.ascontiguousarray(W[3].T).astype(bf)

    in_maps = []
    for c in range(N_CORES):
        r0 = c * HPC * HD
        wqk = np.ascontiguousarray(
            np.concatenate([W[0][r0:r0 + 256], W[1][r0:r0 + 256]], 0).T).astype(bf)
        wv = np.ascontiguousarray(W[2][r0:r0 + 256].T).astype(bf)
        in_maps.append({
            "xT": xT, "wqk": wqk, "wv": wv, "wo": wo,
            "cs": cs, "mask": masks, "ones": ones,
        })
    return in_maps


def kernel(x, W, cos, sin, scale):
    from concourse.bass_utils import run_bass_kernel_spmd

    x = np.asarray(x, dtype=np.float32)
    W = np.asarray(W, dtype=np.float32)
    cos = np.asarray(cos, dtype=np.float32)
    sin = np.asarray(sin, dtype=np.float32)
    sc = float(np.asarray(scale))

    if sc not in _CACHE:
        _CACHE[sc] = _build(sc)
    nc = _CACHE[sc]

    in_maps = _prep_inputs(x, W, cos, sin)
    out = run_bass_kernel_spmd(nc, in_maps, core_ids=list(range(N_CORES)))
    y = np.concatenate([out.results[c]["y"] for c in range(N_CORES)], axis=0)
    return y.reshape(B, T, D)



# revision 2
# speedup vs baseline: 3.4054x; 3.4054x over previous
"""Causal self-attention (B=2, T=2048, D=2048, H=16, HD=128) on 8 TRN2 cores.

The per-iteration cost on this stack is dominated by host->device input
shipping (~1.1 ms per input tensor name + ~0.5 ms per core-MB), not by
on-device compute (~0.5 ms). So the kernel is organized to minimize I/O:

  - ONE packed bf16 input blob per core (~6.8 MB): x feature-row shard
    (256 rows of xT), this core's q/k/v head weights, a 256-column shard
    of W_o (rows permuted to match the AllGather layout), raw cos/sin.
  - On-device AllGather reassembles full xT from the 8 shards.
  - Tensor-parallel attention over heads (2 heads/core): QKV matmul,
    RMS norm, rotary, causal SDPA — all contractions on the partition
    dim, scores computed transposed, softmax denominator via all-ones
    matmul, causal mask via gpsimd affine_select (no mask input).
  - Per-head AllGather of the attention output yT, then a column-
    parallel output projection (each core computes 256 output features
    for all 4096 tokens) — needs only 1 MB of W_o per core instead of
    the full 8 MB.
  - bf16 output [256, 4096] (out-features x tokens), assembled and cast
    to f32 on the host.

Matmuls run in bf16 (fp32 is 4 cycles/row on the PE, bf16 is 1); PSUM
accumulation and softmax statistics stay fp32.
"""

import numpy as np

B, T, D = 2, 2048, 2048
H, HD = 16, 128
N_CORES = 8
HPC = H // N_CORES          # heads per core
NT = B * T                  # 4096 tokens, b-major
DC = D // 128               # 16 contraction chunks
NTT = NT // 512             # 8 token tiles
KT_PER_B = T // 128         # 16 k-tiles per batch row

# packed input blob regions (elements, bf16)
SZ_X = 256 * NT
SZ_WQK = 128 * DC * 512
SZ_WV = 128 * DC * 256
SZ_W3 = 128 * DC * 256
SZ_CS = 64 * T
OFF_X = 0
OFF_WQK = OFF_X + SZ_X
OFF_WV = OFF_WQK + SZ_WQK
OFF_W3 = OFF_WV + SZ_WV
OFF_COS = OFF_W3 + SZ_W3
OFF_SIN = OFF_COS + SZ_CS
BLOB = OFF_SIN + SZ_CS

_CACHE = {}


def _build(scale: float, reps: int = 1):
    import concourse.bacc as bacc
    import concourse.mybir as mybir
    import concourse.tile as tile

    f32 = mybir.dt.float32
    MM = mybir.dt.bfloat16
    EPS = float(np.finfo(np.float32).eps)

    nc = bacc.Bacc("TRN2", target_bir_lowering=False, debug=False,
                   num_devices=N_CORES)

    blob_d = nc.dram_tensor("blob", [BLOB], MM, kind="ExternalInput")
    y_d = nc.dram_tensor("y", [2 * 128, NT], MM, kind="ExternalOutput")

    Sq = mybir.ActivationFunctionType.Square
    Sqrt = mybir.ActivationFunctionType.Sqrt
    Exp = mybir.ActivationFunctionType.Exp
    Copy = mybir.ActivationFunctionType.Copy
    is_ge = mybir.AluOpType.is_ge
    bypass = mybir.AluOpType.bypass
    RG = [list(range(N_CORES))]

    def blob2d(off, p, f):
        return blob_d[off:off + p * f].rearrange("(p f) -> p f", f=f)

    with tile.TileContext(nc) as tc:
        with tc.tile_pool(name="dram", bufs=1, space="DRAM") as dram, \
             tc.tile_pool(name="res", bufs=1) as res:
            agx_in = dram.tile([256, NT], MM, tag="agx_in", name="agx_in")
            agx_out = dram.tile([D, NT], MM, tag="agx_out", name="agx_out",
                                addr_space="Shared")
            agy_in = [dram.tile([128, NT], MM, tag=f"agy_in{h}",
                                name=f"agy_in{h}") for h in range(HPC)]
            agy_out = [dram.tile([1024, NT], MM, tag=f"agy_out{h}",
                                 name=f"agy_out{h}", addr_space="Shared")
                       for h in range(HPC)]

            # residents: rotated q/k (m-chunks q0,q1,k0,k1), v in
            # [token, hd] layout, cos/sin, all-ones, o-proj weights
            qk_sb = res.tile([128, 4 * NT], MM, tag="qk")
            v_sb = res.tile([128, (NT // 128) * (HPC * HD)], MM, tag="v")
            cs_sb = res.tile([128, 2 * T], MM, tag="cs")
            w3_sb = res.tile([128, DC * 256], MM, tag="w3")
            ones_sb = res.tile([128, 128], MM, tag="ones")
            eps_sb = res.tile([128, 1], f32, tag="eps")
            nc.vector.memset(eps_sb[:], EPS)
            nc.vector.memset(ones_sb[:], 1.0)

            for _rep in range(reps):
                # x AllGather first: everything in phase 1 waits on it
                for hf in range(2):
                    nc.sync.dma_start(
                        out=agx_in[hf * 128:(hf + 1) * 128, :],
                        in_=blob2d(OFF_X + hf * 128 * NT, 128, NT))
                nc.gpsimd.collective_compute(
                    "AllGather", bypass, replica_groups=RG,
                    ins=[agx_in.opt()], outs=[agx_out.opt()])

                # cos/sin: C = [cosT; cosT], S = [-sinT; sinT]
                nc.sync.dma_start(out=cs_sb[0:64, 0:T],
                                  in_=blob2d(OFF_COS, 64, T))
                nc.sync.dma_start(out=cs_sb[64:128, 0:T],
                                  in_=blob2d(OFF_COS, 64, T))
                nc.sync.dma_start(out=cs_sb[64:128, T:2 * T],
                                  in_=blob2d(OFF_SIN, 64, T))
                nc.sync.dma_start(out=cs_sb[0:64, T:2 * T],
                                  in_=blob2d(OFF_SIN, 64, T))
                nc.scalar.activation(cs_sb[0:64, T:2 * T],
                                     cs_sb[0:64, T:2 * T], Copy, scale=-1.0)
                nc.sync.dma_start(out=w3_sb[:],
                                  in_=blob2d(OFF_W3, 128, DC * 256))

                # ---------------- Phase 1: QKV + RMS norm + rotary ----------------
                with tc.tile_pool(name="p1", bufs=1) as p1, \
                     tc.tile_pool(name="xs", bufs=3) as xs, \
                     tc.tile_pool(name="st", bufs=3) as st, \
                     tc.tile_pool(name="ps1", bufs=2, space="PSUM") as ps1:
                    wqk_sb = p1.tile([128, DC * 512], MM, tag="wqk")
                    wv_sb = p1.tile([128, DC * 256], MM, tag="wv")
                    nc.sync.dma_start(out=wv_sb[:],
                                      in_=blob2d(OFF_WV, 128, DC * 256))
                    nc.sync.dma_start(out=wqk_sb[:],
                                      in_=blob2d(OFF_WQK, 128, DC * 512))

                    for n in range(NTT):
                        xblk = xs.tile([128, DC * 512], MM, tag="xblk")
                        for cg in range(4):
                            nc.sync.dma_start(
                                out=xblk[:, cg * 4 * 512:(cg + 1) * 4 * 512]
                                    .rearrange("p (c f) -> p c f", f=512),
                                in_=agx_out[cg * 512:(cg + 1) * 512,
                                            n * 512:(n + 1) * 512]
                                    .rearrange("(c p) f -> p c f", p=128))
                        # v projection: [token, hd] layout
                        for c4 in range(4):
                            vps = ps1.tile([128, HPC * HD], f32, tag="vps")
                            for dc in range(DC):
                                nc.tensor.matmul(
                                    vps[:],
                                    xblk[:, dc * 512 + c4 * 128: dc * 512 + (c4 + 1) * 128],
                                    wv_sb[:, dc * 256:(dc + 1) * 256],
                                    start=(dc == 0), stop=(dc == DC - 1))
                            tcg = n * 4 + c4
                            nc.vector.tensor_copy(v_sb[:, tcg * 256:(tcg + 1) * 256], vps[:])
                        # q/k projection + rms + rotary, m-chunks q0,q1,k0,k1
                        for m in range(4):
                            qps = ps1.tile([128, 512], f32, tag="qps")
                            for dc in range(DC):
                                nc.tensor.matmul(
                                    qps[:],
                                    wqk_sb[:, dc * 512 + m * 128: dc * 512 + (m + 1) * 128],
                                    xblk[:, dc * 512:(dc + 1) * 512],
                                    start=(dc == 0), stop=(dc == DC - 1))
                            sq = st.tile([128, 512], MM, tag="sq")
                            nc.scalar.activation(sq[:], qps[:], Sq)
                            ssq = ps1.tile([128, 512], f32, tag="ssq")
                            nc.tensor.matmul(ssq[:], ones_sb[:], sq[:], start=True, stop=True)
                            rms = st.tile([128, 512], f32, tag="rms")
                            nc.scalar.activation(rms[:], ssq[:], Sqrt, bias=eps_sb[:], scale=1.0 / HD)
                            r = st.tile([128, 512], f32, tag="r")
                            nc.vector.reciprocal(r[:], rms[:])
                            qn = st.tile([128, 512], MM, tag="qn")
                            nc.vector.tensor_mul(qn[:], qps[:], r[:])
                            # rotary: y = qn*C + swap(qn)*S  with S = [-sin; sin]
                            tsw = st.tile([128, 512], MM, tag="tsw")
                            tb = (n % 4) * 512
                            ctile = cs_sb[:, tb:tb + 512]
                            stile = cs_sb[:, T + tb:T + tb + 512]
                            nc.vector.tensor_mul(tsw[0:64, :], qn[64:128, :], stile[64:128, :])
                            nc.vector.tensor_mul(tsw[64:128, :], qn[0:64, :], stile[0:64, :])
                            dst = qk_sb[:, m * NT + n * 512: m * NT + (n + 1) * 512]
                            nc.vector.tensor_mul(dst, qn[:], ctile)
                            nc.vector.tensor_add(dst, dst, tsw[:])

                # ---------------- Phase 2: attention + per-head AllGather ----------------
                with tc.tile_pool(name="p2", bufs=4) as p2, \
                     tc.tile_pool(name="p2b", bufs=2) as p2b, \
                     tc.tile_pool(name="pss", bufs=2, space="PSUM") as pss, \
                     tc.tile_pool(name="psd", bufs=2, space="PSUM") as psd, \
                     tc.tile_pool(name="psy", bufs=2, space="PSUM") as psy:
                    for h in range(HPC):
                        qoff = h * NT
                        koff = (2 + h) * NT
                        for b in range(B):
                            for qj in range(4):
                                yps = psy.tile([128, 512], f32, tag="yps")
                                dps = psd.tile([128, 512], f32, tag="dps")
                                nkt = 4 * qj + 4
                                qbase = qoff + b * T + qj * 512
                                for kb in range(nkt):
                                    # diagonal blocks: only q-columns >= 128*m live
                                    lo = max(0, (kb - 4 * qj) * 128)
                                    sps = pss.tile([128, 512], f32, tag="sps")
                                    nc.tensor.matmul(
                                        sps[:, lo:],
                                        qk_sb[:, koff + b * T + kb * 128: koff + b * T + (kb + 1) * 128],
                                        qk_sb[:, qbase + lo: qbase + 512],
                                        start=True, stop=True)
                                    e = p2.tile([128, 512], MM, tag="e")
                                    nc.scalar.activation(e[:, lo:], sps[:, lo:], Exp, scale=scale)
                                    if kb >= 4 * qj:
                                        # causal: keep col j (>=lo) iff local
                                        # k-row p <= j - lo  (j - p - lo >= 0)
                                        nc.gpsimd.affine_select(
                                            out=e[:, lo:], in_=e[:, lo:],
                                            pattern=[[1, 512 - lo]],
                                            compare_op=is_ge, fill=0.0,
                                            base=0, channel_multiplier=-1)
                                    nc.tensor.matmul(dps[:, lo:], ones_sb[:], e[:, lo:],
                                                     start=(kb == 0), stop=(kb == nkt - 1))
                                    tcg = b * KT_PER_B + kb
                                    nc.tensor.matmul(
                                        yps[:, lo:],
                                        v_sb[:, tcg * 256 + h * 128: tcg * 256 + (h + 1) * 128],
                                        e[:, lo:],
                                        start=(kb == 0), stop=(kb == nkt - 1))
                                rcp = p2b.tile([128, 512], f32, tag="rcp")
                                nc.vector.reciprocal(rcp[:], dps[:])
                                yn = p2b.tile([128, 512], MM, tag="yn")
                                nc.vector.tensor_mul(yn[:], yps[:], rcp[:])
                                s = b * 4 + qj
                                nc.sync.dma_start(
                                    out=agy_in[h][:, s * 512:(s + 1) * 512],
                                    in_=yn[:])
                        # head h's AllGather overlaps head h+1's attention
                        nc.gpsimd.collective_compute(
                            "AllGather", bypass, replica_groups=RG,
                            ins=[agy_in[h].opt()], outs=[agy_out[h].opt()])

                # ---------------- Phase 3: column-parallel o-proj ----------------
                # ytile d-chunk dc: rows of agy_out[dc//8] block (dc%8);
                # w3 host layout is permuted to match.
                with tc.tile_pool(name="yt", bufs=2) as ytp, \
                     tc.tile_pool(name="ob", bufs=2) as obp, \
                     tc.tile_pool(name="ps3", bufs=2, space="PSUM") as ps3:
                    for tt in range(NTT):
                        ytile = ytp.tile([128, DC * 512], MM, tag="ytile")
                        for h in range(HPC):
                            nc.sync.dma_start(
                                out=ytile[:, h * 8 * 512:(h + 1) * 8 * 512]
                                    .rearrange("p (c f) -> p c f", f=512),
                                in_=agy_out[h][0:1024, tt * 512:(tt + 1) * 512]
                                    .rearrange("(c p) f -> p c f", p=128))
                        for oc2 in range(2):
                            ps = ps3.tile([128, 512], f32, tag="ops")
                            for dc in range(DC):
                                nc.tensor.matmul(
                                    ps[:],
                                    w3_sb[:, dc * 256 + oc2 * 128: dc * 256 + (oc2 + 1) * 128],
                                    ytile[:, dc * 512:(dc + 1) * 512],
                                    start=(dc == 0), stop=(dc == DC - 1))
                            ob = obp.tile([128, 512], MM, tag="ob")
                            nc.scalar.activation(ob[:], ps[:], Copy)
                            nc.sync.dma_start(
                                out=y_d[oc2 * 128:(oc2 + 1) * 128,
                                        tt * 512:(tt + 1) * 512],
                                in_=ob[:])

    nc.compile()
    return nc


def _prep_inputs(x, W, cos, sin):
    import concourse.mybir as mybir
    bf = mybir.dt.np(mybir.dt.bfloat16)

    xT = np.ascontiguousarray(x.reshape(NT, D).T).astype(bf)  # [D, NT]
    cosT = cos.T.astype(bf)  # [64, T]
    sinT = sin.T.astype(bf)
    W3T = W[3].T  # [d_in, d_out]
    # AllGather row order: AG#h stacks head (2c+h) of core c at block c
    blocks = [2 * c for c in range(N_CORES)] + [2 * c + 1 for c in range(N_CORES)]
    rows = np.concatenate([np.arange(b * 128, (b + 1) * 128) for b in blocks])
    W3p = W3T[rows]  # [d_in permuted, d_out]

    in_maps = []
    for c in range(N_CORES):
        r0 = c * HPC * HD
        wqk = np.concatenate([W[0][r0:r0 + 256], W[1][r0:r0 + 256]], 0).T  # [D, 512]
        wqk_sb = wqk.reshape(DC, 128, 512).transpose(1, 0, 2)
        wv = W[2][r0:r0 + 256].T  # [D, 256]
        wv_sb = wv.reshape(DC, 128, 256).transpose(1, 0, 2)
        w3_sb = W3p[:, r0:r0 + 256].reshape(DC, 128, 256).transpose(1, 0, 2)

        blob = np.empty(BLOB, bf)
        blob[OFF_X:OFF_X + SZ_X] = xT[c * 256:(c + 1) * 256].reshape(-1)
        blob[OFF_WQK:OFF_WQK + SZ_WQK] = wqk_sb.astype(bf).reshape(128, -1).reshape(-1)
        blob[OFF_WV:OFF_WV + SZ_WV] = wv_sb.astype(bf).reshape(128, -1).reshape(-1)
        blob[OFF_W3:OFF_W3 + SZ_W3] = w3_sb.astype(bf).reshape(128, -1).reshape(-1)
        blob[OFF_COS:OFF_COS + SZ_CS] = cosT.reshape(-1)
        blob[OFF_SIN:OFF_SIN + SZ_CS] = sinT.reshape(-1)
        in_maps.append({"blob": blob})
    return in_maps


def kernel(x, W, cos, sin, scale):
    from concourse.bass_utils import run_bass_kernel_spmd

    x = np.asarray(x, dtype=np.float32)
    W = np.asarray(W, dtype=np.float32)
    cos = np.asarray(cos, dtype=np.float32)
    sin = np.asarray(sin, dtype=np.float32)
    sc = float(np.asarray(scale))

    if sc not in _CACHE:
        _CACHE[sc] = _build(sc)
    nc = _CACHE[sc]

    in_maps = _prep_inputs(x, W, cos, sin)
    out = run_bass_kernel_spmd(nc, in_maps, core_ids=list(range(N_CORES)))
    yT = np.concatenate(
        [np.asarray(out.results[c]["y"], dtype=np.float32) for c in range(N_CORES)],
        axis=0)  # [D, NT]
    return np.ascontiguousarray(yT.T).reshape(B, T, D)
